# revision 1
# baseline (speedup 1.0000x reference)
"""Trainium2 Bass kernel: 3x3 valid cross-correlation (dense CNN layer).

  x:       (128, 224, 224) f32   (C_in, H, W)
  kernels: (256, 128, 3, 3) f32  (C_out, C_in, KH, KW)
  out:     (256, 222, 222) f32   (C_out, H_out, W_out)

Sharding: output rows spatially across the 8 NeuronCores (28 rows per core;
8*28 = 224 >= 222, tail rows computed from zero padding and dropped on
gather). Every core holds the full filter bank. C_in = 128 is exactly the PE
contraction dim; output channels form two 128-partition groups. For each
(row-pair, channel-group) a PSUM tile (128, 2, 222) accumulates one matmul
per filter tap, the moving operand being a shifted window of the SBUF-
resident input slab.

Precision modes (CONV_MM_MODE):
  f32r3 (default): fp32r hi/lo split. fp32r is fp32 RNE-rounded to 11
      explicit mantissa bits but streams at full PE rate; the matmul is
      exact (fp32 PSUM accumulate) on fp32r-representable values. With
      x = x_hi + x_lo and w = w_hi + w_lo (both halves exactly
      representable), 3 passes (hi*hi + hi*lo + lo*hi) give full-fp32
      accuracy at 3 cycles/row vs plain fp32's 4.
  f32r: single-pass fp32r (~1.5e-4 rel err, 1 cycle/row).
  f32:  plain fp32 matmul (4 cycles/row).
  bf16: single-pass bf16 (~1e-2 rel err, 1 cycle/row).
"""

import os
from contextlib import ExitStack

import numpy as np

C_IN, H, W = 128, 224, 224
C_OUT, KH, KW = 256, 3, 3
H_OUT = H - KH + 1  # 222
W_OUT = W - KW + 1  # 222
N_CORES = 8
ROWS_PER_CORE = 28
IN_ROWS = ROWS_PER_CORE + KH - 1  # 30
PAIRS = ROWS_PER_CORE // 2  # 14
N_GROUPS = C_OUT // 128  # 2
H_PAD = N_CORES * ROWS_PER_CORE + KH - 1  # 226
N_TAPS = KH * KW  # 9

MM_MODE = os.environ.get("CONV_MM_MODE", "f32r3")

_compiled = {}


def _round_f32r(a):
    """fp32 -> nearest fp32r (low 12 mantissa bits dropped, RNE) — the exact
    rounding trn2 applies when data is ingested as dt.float32r."""
    b = a.view(np.uint32).astype(np.uint64)
    q = np.uint64(1 << 12)
    r = (b + ((q >> np.uint64(1)) - np.uint64(1) + ((b >> np.uint64(12)) & np.uint64(1)))) & ~(q - np.uint64(1))
    return r.astype(np.uint32).view(np.float32)


DEFAULT_CFG = dict(
    xp_bufs=1,
    wp_bufs=1,
    op_bufs=8,
    pp_bufs=8,
    # term-major matmul order + interleaved hi/lo input chunks + per-group w
    # chunks minimize the pipeline-fill stall at kernel start (the first 9
    # matmuls only need w_hi[g0] and the first x chunks). Sustained slope is
    # PE-bound and config-insensitive; these help the single-shot case.
    term_major=True,
    x_chunk=6,
    w_group_chunks=True,
    x_h_outer=False,
    in_dma_gpsimd=False,  # issue input DMAs from gpsimd (separate queues from output)
    pair_block=0,  # >0: tap-major over a block of row-pairs sharing each weight
    # ablation flags (repeat-loop timing experiments)
    load_in_loop=True,  # False: hoist x/w DMA out of the repeat loop
    do_copy=True,  # False: skip psum->sbuf copy except an anchor on the last tile
    do_store=True,  # False: skip output DMA
)


def _build(mm_mode, repeat=1, **cfg_over):
    import concourse.mybir as mybir
    import concourse.tile as tile
    from concourse import bacc

    cfg = {**DEFAULT_CFG, **cfg_over}
    dt = mybir.dt
    split = mm_mode == "f32r3"
    mm_dt = {
        "f32r3": dt.float32r,
        "f32r": dt.float32r,
        "f32": dt.float32,
        "bf16": dt.bfloat16,
    }[mm_mode]
    n_half = 2 if split else 1  # hi/lo copies of x and w

    nc = bacc.Bacc("TRN2", target_bir_lowering=False)
    x_d = nc.dram_tensor(
        "x", [n_half, C_IN, IN_ROWS, W], mm_dt, kind="ExternalInput"
    ).ap()
    w_d = nc.dram_tensor(
        "w", [n_half, C_IN, N_GROUPS * N_TAPS, 128], mm_dt, kind="ExternalInput"
    ).ap()
    o_d = nc.dram_tensor(
        "out", [N_GROUPS, 128, ROWS_PER_CORE, W_OUT], dt.float32, kind="ExternalOutput"
    ).ap()

    def load(nc, tc, xp, wp):
        in_eng = nc.gpsimd if cfg["in_dma_gpsimd"] else nc.sync
        w_sb = wp.tile([C_IN, n_half * N_GROUPS * N_TAPS, 128], mm_dt, name="w_sb")
        if cfg["w_group_chunks"]:
            for h in range(n_half):
                for g in range(N_GROUPS):
                    in_eng.dma_start(
                        w_sb[
                            :,
                            h * N_GROUPS * N_TAPS + g * N_TAPS : h * N_GROUPS * N_TAPS
                            + (g + 1) * N_TAPS,
                            :,
                        ],
                        w_d[h, :, g * N_TAPS : (g + 1) * N_TAPS, :],
                    )
        else:
            for h in range(n_half):
                in_eng.dma_start(
                    w_sb[:, h * N_GROUPS * N_TAPS : (h + 1) * N_GROUPS * N_TAPS, :],
                    w_d[h],
                )
        x_sb = xp.tile([C_IN, n_half * IN_ROWS, W], mm_dt, name="x_sb")
        x_chunk = cfg["x_chunk"]
        if cfg["x_h_outer"]:
            for h in range(n_half):
                for r0 in range(0, IN_ROWS, x_chunk):
                    r1 = min(r0 + x_chunk, IN_ROWS)
                    in_eng.dma_start(
                        x_sb[:, h * IN_ROWS + r0 : h * IN_ROWS + r1, :],
                        x_d[h, :, r0:r1, :],
                    )
        else:
            for r0 in range(0, IN_ROWS, x_chunk):
                r1 = min(r0 + x_chunk, IN_ROWS)
                for h in range(n_half):
                    in_eng.dma_start(
                        x_sb[:, h * IN_ROWS + r0 : h * IN_ROWS + r1, :],
                        x_d[h, :, r0:r1, :],
                    )
        return w_sb, x_sb

    def compute(nc, tc, op, pp, w_sb, x_sb):
        # matmul passes per tap: (w_half, x_half)
        terms = [(0, 0), (0, 1), (1, 0)] if split else [(0, 0)]
        n_mm = len(terms) * N_TAPS
        taps = [(kh, kw) for kh in range(KH) for kw in range(KW)]
        if cfg["term_major"]:
            mm_order = [(wh, xh, kh, kw) for (wh, xh) in terms for (kh, kw) in taps]
        else:
            mm_order = [(wh, xh, kh, kw) for (kh, kw) in taps for (wh, xh) in terms]

        def emit_mm(ps, p, g, wh, xh, kh, kw, start, stop):
            nc.tensor.matmul(
                ps[:],
                w_sb[:, wh * N_GROUPS * N_TAPS + (g * KH + kh) * KW + kw, :],
                x_sb[
                    :,
                    xh * IN_ROWS + 2 * p + kh : xh * IN_ROWS + 2 * p + kh + 2,
                    kw : kw + W_OUT,
                ],
                start=start,
                stop=stop,
            )

        def emit_out(ps, p, g, last):
            if cfg["do_copy"] or last:
                ot = op.tile([128, 2, W_OUT], dt.float32, name="ot")
                nc.vector.tensor_copy(ot[:], ps[:])
                if cfg["do_store"] or last:
                    nc.sync.dma_start(o_d[g, :, 2 * p : 2 * p + 2, :], ot[:])

        B = cfg["pair_block"]
        if B:
            # consecutive matmuls share one stationary weight across B pairs
            for g in range(N_GROUPS):
                for b0 in range(0, PAIRS, B):
                    blk = list(range(b0, min(b0 + B, PAIRS)))
                    tiles = {
                        p: pp.tile([128, 2, W_OUT], dt.float32, name="ps") for p in blk
                    }
                    for i_mm, (wh, xh, kh, kw) in enumerate(mm_order):
                        for p in blk:
                            emit_mm(
                                tiles[p], p, g, wh, xh, kh, kw,
                                i_mm == 0, i_mm == n_mm - 1,
                            )
                    for p in blk:
                        emit_out(
                            tiles[p], p, g,
                            p == PAIRS - 1 and g == N_GROUPS - 1,
                        )
        else:
            for p in range(PAIRS):
                for g in range(N_GROUPS):
                    ps = pp.tile([128, 2, W_OUT], dt.float32, name="ps")
                    for i_mm, (wh, xh, kh, kw) in enumerate(mm_order):
                        emit_mm(ps, p, g, wh, xh, kh, kw, i_mm == 0, i_mm == n_mm - 1)
                    emit_out(ps, p, g, p == PAIRS - 1 and g == N_GROUPS - 1)

    with tile.TileContext(nc) as tc, ExitStack() as ctx:
        xp = ctx.enter_context(tc.tile_pool(name="xp", bufs=cfg["xp_bufs"]))
        wp = ctx.enter_context(tc.tile_pool(name="wp", bufs=cfg["wp_bufs"]))
        op = ctx.enter_context(tc.tile_pool(name="op", bufs=cfg["op_bufs"]))
        pp = ctx.enter_context(
            tc.tile_pool(name="pp", bufs=cfg["pp_bufs"], space="PSUM")
        )
        if repeat == 1:
            w_sb, x_sb = load(nc, tc, xp, wp)
            compute(nc, tc, op, pp, w_sb, x_sb)
        elif cfg["load_in_loop"]:
            with tc.For_i(0, repeat, 1):
                w_sb, x_sb = load(nc, tc, xp, wp)
                compute(nc, tc, op, pp, w_sb, x_sb)
        else:
            w_sb, x_sb = load(nc, tc, xp, wp)
            with tc.For_i(0, repeat, 1):
                compute(nc, tc, op, pp, w_sb, x_sb)

    nc.compile()
    return nc


def _get_nc(mode):
    if mode not in _compiled:
        _compiled[mode] = _build(mode)
    return _compiled[mode]


def _prep_inputs(x, kernels, mode):
    x = np.asarray(x, dtype=np.float32)
    kernels = np.asarray(kernels, dtype=np.float32)
    x_pad = np.zeros((C_IN, H_PAD, W), np.float32)
    x_pad[:, :H, :] = x
    # lhsT layout: [cin, (group kh kw), cout_in_group]
    w = kernels.reshape(N_GROUPS, 128, C_IN, KH, KW).transpose(2, 0, 3, 4, 1)
    w = np.ascontiguousarray(w).reshape(C_IN, N_GROUPS * N_TAPS, 128)

    if mode == "f32r3":
        x_hi = _round_f32r(x_pad)
        x_lo = x_pad - x_hi
        w_hi = _round_f32r(w)
        w_lo = w - w_hi
        xs = np.stack([x_hi, x_lo])  # (2, C_IN, H_PAD, W)
        ws = np.stack([w_hi, w_lo])  # (2, C_IN, 18, 128)
    else:
        xs = x_pad[None]
        ws = w[None]
        if mode == "bf16":
            import ml_dtypes

            xs = xs.astype(ml_dtypes.bfloat16)
            ws = ws.astype(ml_dtypes.bfloat16)

    in_maps = [
        {
            "x": np.ascontiguousarray(
                xs[:, :, ROWS_PER_CORE * i : ROWS_PER_CORE * i + IN_ROWS, :]
            ),
            "w": ws,
        }
        for i in range(N_CORES)
    ]
    return in_maps


def _gather(results):
    out = np.empty((C_OUT, N_CORES * ROWS_PER_CORE, W_OUT), np.float32)
    for i in range(N_CORES):
        o = results[i]["out"]  # (2, 128, 28, 222)
        r0 = ROWS_PER_CORE * i
        out[:128, r0 : r0 + ROWS_PER_CORE, :] = o[0]
        out[128:, r0 : r0 + ROWS_PER_CORE, :] = o[1]
    return np.ascontiguousarray(out[:, :H_OUT, :])


def _run(x, kernels, mode=None, **spmd_kwargs):
    from concourse.bass_utils import run_bass_kernel_spmd

    mode = mode or MM_MODE
    nc = _get_nc(mode)
    in_maps = _prep_inputs(x, kernels, mode)
    res = run_bass_kernel_spmd(nc, in_maps, list(range(N_CORES)), **spmd_kwargs)
    return _gather(res.results), res


def kernel(x, kernels):
    out, _ = _run(x, kernels)
    return out



# revision 2
# speedup vs baseline: 1.6891x; 1.6891x over previous
"""Trainium2 Bass kernel: 3x3 valid cross-correlation (dense CNN layer).

  x:       (128, 224, 224) f32   (C_in, H, W)
  kernels: (256, 128, 3, 3) f32  (C_out, C_in, KH, KW)
  out:     (256, 222, 222) f32   (C_out, H_out, W_out)

Sharding: output rows spatially across the 8 NeuronCores (28 rows per core;
8*28 = 224 >= 222, tail rows computed from zero padding and dropped on
gather). Every core holds the full filter bank. C_in = 128 is exactly the PE
contraction dim; output channels form two 128-partition groups. For each
(row-pair, channel-group) a PSUM tile (128, 2, 222) accumulates one matmul
per filter tap, the moving operand being a shifted window of the SBUF-
resident input slab.

Precision modes (CONV_MM_MODE):
  mix (default): stationary weights in bf16 (1-pass LDWEIGHTS that the PE
      reorder window can hide behind the previous matmul), moving x in fp32r
      (full-rate streaming). ~1e-3 rel err from the bf16 weight quantization.
  f32r: single-pass fp32r both operands (~1.5e-4 rel err) but the 4-byte
      stationary load is 2 LDW passes and not hidden.
  f32r3: fp32r hi/lo split, 3 matmul passes, full-fp32 accuracy.
  f32:  plain fp32 matmul (4 cycles/row).
  bf16: single-pass bf16 (~1e-2 rel err).
"""

import os
from contextlib import ExitStack

import numpy as np

C_IN, H, W = 128, 224, 224
C_OUT, KH, KW = 256, 3, 3
H_OUT = H - KH + 1  # 222
W_OUT = W - KW + 1  # 222
N_CORES = 8
ROWS_PER_CORE = 28
IN_ROWS = ROWS_PER_CORE + KH - 1  # 30
PAIRS = ROWS_PER_CORE // 2  # 14
N_GROUPS = C_OUT // 128  # 2
H_PAD = N_CORES * ROWS_PER_CORE + KH - 1  # 226
N_TAPS = KH * KW  # 9

MM_MODE = os.environ.get("CONV_MM_MODE", "mix")

_compiled = {}


def _round_f32r(a):
    """fp32 -> nearest fp32r (low 12 mantissa bits dropped, RNE) — the exact
    rounding trn2 applies when data is ingested as dt.float32r."""
    b = a.view(np.uint32).astype(np.uint64)
    q = np.uint64(1 << 12)
    r = (b + ((q >> np.uint64(1)) - np.uint64(1) + ((b >> np.uint64(12)) & np.uint64(1)))) & ~(q - np.uint64(1))
    return r.astype(np.uint32).view(np.float32)


# (x_dtype, w_dtype, n_split_halves)
MODE_DT = {
    "f32r3": ("float32r", "float32r", 2),
    "f32r": ("float32r", "float32r", 1),
    "mix": ("float32r", "bfloat16", 1),
    "bf16": ("bfloat16", "bfloat16", 1),
    "f32": ("float32", "float32", 1),
}

DEFAULT_CFG = dict(
    xp_bufs=2,
    wp_bufs=2,
    op_bufs=8,
    pp_bufs=8,
    # term-major matmul order + interleaved hi/lo input chunks + per-group w
    # chunks minimize the pipeline-fill stall at kernel start (the first 9
    # matmuls only need w_hi[g0] and the first x chunks). Sustained slope is
    # PE-bound and config-insensitive; these help the single-shot case.
    term_major=True,
    x_chunk=6,
    w_group_chunks=True,
    x_h_outer=False,
    in_dma_gpsimd=False,  # issue input DMAs from gpsimd (separate queues from output)
    pair_block=0,  # >0: tap-major over a block of row-pairs sharing each weight
    # ablation flags (repeat-loop timing experiments)
    load_in_loop=True,  # False: hoist x/w DMA out of the repeat loop
    do_copy=True,  # False: skip psum->sbuf copy except an anchor on the last tile
    do_store=True,  # False: skip output DMA
)


def _build(mm_mode, repeat=1, **cfg_over):
    import concourse.mybir as mybir
    import concourse.tile as tile
    from concourse import bacc

    cfg = {**DEFAULT_CFG, **cfg_over}
    dt = mybir.dt
    x_dtn, w_dtn, n_half = MODE_DT[mm_mode]
    split = n_half == 2
    x_dt = getattr(dt, x_dtn)
    w_dt = getattr(dt, w_dtn)

    nc = bacc.Bacc("TRN2", target_bir_lowering=False)
    x_d = nc.dram_tensor(
        "x", [n_half, C_IN, IN_ROWS, W], x_dt, kind="ExternalInput"
    ).ap()
    w_d = nc.dram_tensor(
        "w", [n_half, C_IN, N_GROUPS * N_TAPS, 128], w_dt, kind="ExternalInput"
    ).ap()
    o_d = nc.dram_tensor(
        "out", [N_GROUPS, 128, ROWS_PER_CORE, W_OUT], dt.float32, kind="ExternalOutput"
    ).ap()

    def load(nc, tc, xp, wp):
        in_eng = nc.gpsimd if cfg["in_dma_gpsimd"] else nc.sync
        w_sb = wp.tile([C_IN, n_half * N_GROUPS * N_TAPS, 128], w_dt, name="w_sb")
        if cfg["w_group_chunks"]:
            for h in range(n_half):
                for g in range(N_GROUPS):
                    in_eng.dma_start(
                        w_sb[
                            :,
                            h * N_GROUPS * N_TAPS + g * N_TAPS : h * N_GROUPS * N_TAPS
                            + (g + 1) * N_TAPS,
                            :,
                        ],
                        w_d[h, :, g * N_TAPS : (g + 1) * N_TAPS, :],
                    )
        else:
            for h in range(n_half):
                in_eng.dma_start(
                    w_sb[:, h * N_GROUPS * N_TAPS : (h + 1) * N_GROUPS * N_TAPS, :],
                    w_d[h],
                )
        x_sb = xp.tile([C_IN, n_half * IN_ROWS, W], x_dt, name="x_sb")
        x_chunk = cfg["x_chunk"]
        if cfg["x_h_outer"]:
            for h in range(n_half):
                for r0 in range(0, IN_ROWS, x_chunk):
                    r1 = min(r0 + x_chunk, IN_ROWS)
                    in_eng.dma_start(
                        x_sb[:, h * IN_ROWS + r0 : h * IN_ROWS + r1, :],
                        x_d[h, :, r0:r1, :],
                    )
        else:
            for r0 in range(0, IN_ROWS, x_chunk):
                r1 = min(r0 + x_chunk, IN_ROWS)
                for h in range(n_half):
                    in_eng.dma_start(
                        x_sb[:, h * IN_ROWS + r0 : h * IN_ROWS + r1, :],
                        x_d[h, :, r0:r1, :],
                    )
        return w_sb, x_sb

    def compute(nc, tc, op, pp, w_sb, x_sb):
        # matmul passes per tap: (w_half, x_half)
        terms = [(0, 0), (0, 1), (1, 0)] if split else [(0, 0)]
        n_mm = len(terms) * N_TAPS
        taps = [(kh, kw) for kh in range(KH) for kw in range(KW)]
        if cfg["term_major"]:
            mm_order = [(wh, xh, kh, kw) for (wh, xh) in terms for (kh, kw) in taps]
        else:
            mm_order = [(wh, xh, kh, kw) for (kh, kw) in taps for (wh, xh) in terms]

        def emit_mm(ps, p, g, wh, xh, kh, kw, start, stop):
            nc.tensor.matmul(
                ps[:],
                w_sb[:, wh * N_GROUPS * N_TAPS + (g * KH + kh) * KW + kw, :],
                x_sb[
                    :,
                    xh * IN_ROWS + 2 * p + kh : xh * IN_ROWS + 2 * p + kh + 2,
                    kw : kw + W_OUT,
                ],
                start=start,
                stop=stop,
            )

        def emit_out(ps, p, g, last):
            if cfg["do_copy"] or last:
                ot = op.tile([128, 2, W_OUT], dt.float32, name="ot")
                nc.vector.tensor_copy(ot[:], ps[:])
                if cfg["do_store"] or last:
                    nc.sync.dma_start(o_d[g, :, 2 * p : 2 * p + 2, :], ot[:])

        B = cfg["pair_block"]
        if B:
            # consecutive matmuls share one stationary weight across B pairs
            for g in range(N_GROUPS):
                for b0 in range(0, PAIRS, B):
                    blk = list(range(b0, min(b0 + B, PAIRS)))
                    tiles = {
                        p: pp.tile([128, 2, W_OUT], dt.float32, name="ps") for p in blk
                    }
                    for i_mm, (wh, xh, kh, kw) in enumerate(mm_order):
                        for p in blk:
                            emit_mm(
                                tiles[p], p, g, wh, xh, kh, kw,
                                i_mm == 0, i_mm == n_mm - 1,
                            )
                    for p in blk:
                        emit_out(
                            tiles[p], p, g,
                            p == PAIRS - 1 and g == N_GROUPS - 1,
                        )
        else:
            for p in range(PAIRS):
                for g in range(N_GROUPS):
                    ps = pp.tile([128, 2, W_OUT], dt.float32, name="ps")
                    for i_mm, (wh, xh, kh, kw) in enumerate(mm_order):
                        emit_mm(ps, p, g, wh, xh, kh, kw, i_mm == 0, i_mm == n_mm - 1)
                    emit_out(ps, p, g, p == PAIRS - 1 and g == N_GROUPS - 1)

    with tile.TileContext(nc) as tc, ExitStack() as ctx:
        xp = ctx.enter_context(tc.tile_pool(name="xp", bufs=cfg["xp_bufs"]))
        wp = ctx.enter_context(tc.tile_pool(name="wp", bufs=cfg["wp_bufs"]))
        op = ctx.enter_context(tc.tile_pool(name="op", bufs=cfg["op_bufs"]))
        pp = ctx.enter_context(
            tc.tile_pool(name="pp", bufs=cfg["pp_bufs"], space="PSUM")
        )
        if repeat == 1:
            w_sb, x_sb = load(nc, tc, xp, wp)
            compute(nc, tc, op, pp, w_sb, x_sb)
        elif cfg["load_in_loop"]:
            with tc.For_i(0, repeat, 1):
                w_sb, x_sb = load(nc, tc, xp, wp)
                compute(nc, tc, op, pp, w_sb, x_sb)
        else:
            w_sb, x_sb = load(nc, tc, xp, wp)
            with tc.For_i(0, repeat, 1):
                compute(nc, tc, op, pp, w_sb, x_sb)

    nc.compile()
    return nc


def _get_nc(mode):
    if mode not in _compiled:
        _compiled[mode] = _build(mode)
    return _compiled[mode]


def _prep_inputs(x, kernels, mode):
    x = np.asarray(x, dtype=np.float32)
    kernels = np.asarray(kernels, dtype=np.float32)
    x_pad = np.zeros((C_IN, H_PAD, W), np.float32)
    x_pad[:, :H, :] = x
    # lhsT layout: [cin, (group kh kw), cout_in_group]
    w = kernels.reshape(N_GROUPS, 128, C_IN, KH, KW).transpose(2, 0, 3, 4, 1)
    w = np.ascontiguousarray(w).reshape(C_IN, N_GROUPS * N_TAPS, 128)

    if mode == "f32r3":
        x_hi = _round_f32r(x_pad)
        x_lo = x_pad - x_hi
        w_hi = _round_f32r(w)
        w_lo = w - w_hi
        xs = np.stack([x_hi, x_lo])  # (2, C_IN, H_PAD, W)
        ws = np.stack([w_hi, w_lo])  # (2, C_IN, 18, 128)
    else:
        xs = x_pad[None]
        ws = w[None]
        if mode in ("bf16", "mix"):
            import ml_dtypes

            ws = ws.astype(ml_dtypes.bfloat16)
        if mode == "bf16":
            import ml_dtypes

            xs = xs.astype(ml_dtypes.bfloat16)

    in_maps = [
        {
            "x": np.ascontiguousarray(
                xs[:, :, ROWS_PER_CORE * i : ROWS_PER_CORE * i + IN_ROWS, :]
            ),
            "w": ws,
        }
        for i in range(N_CORES)
    ]
    return in_maps


def _gather(results):
    out = np.empty((C_OUT, N_CORES * ROWS_PER_CORE, W_OUT), np.float32)
    for i in range(N_CORES):
        o = results[i]["out"]  # (2, 128, 28, 222)
        r0 = ROWS_PER_CORE * i
        out[:128, r0 : r0 + ROWS_PER_CORE, :] = o[0]
        out[128:, r0 : r0 + ROWS_PER_CORE, :] = o[1]
    return np.ascontiguousarray(out[:, :H_OUT, :])


def _run(x, kernels, mode=None, **spmd_kwargs):
    from concourse.bass_utils import run_bass_kernel_spmd

    mode = mode or MM_MODE
    nc = _get_nc(mode)
    in_maps = _prep_inputs(x, kernels, mode)
    res = run_bass_kernel_spmd(nc, in_maps, list(range(N_CORES)), **spmd_kwargs)
    return _gather(res.results), res


def kernel(x, kernels):
    out, _ = _run(x, kernels)
    return out


# revision 6
# speedup vs baseline: 5.5140x; 3.2645x over previous
"""Trainium2 Bass kernel: 3x3 valid cross-correlation (dense CNN layer).

  x:       (128, 224, 224) f32   (C_in, H, W)
  kernels: (256, 128, 3, 3) f32  (C_out, C_in, KH, KW)
  out:     (256, 222, 222) f32   (C_out, H_out, W_out)

Sharding: output rows spatially across the 8 NeuronCores (28 rows per core;
8*28 = 224 >= 222, tail rows computed from zero padding and dropped on
gather). Every core holds the full filter bank. C_in = 128 is exactly the PE
contraction dim; output channels form two 128-partition groups. For each
(row-pair, channel-group) a PSUM tile (128, 2, 222) accumulates one matmul
per filter tap, the moving operand being a shifted window of the SBUF-
resident input slab.

Precision modes (CONV_MM_MODE):
  f16 (default): single-pass fp16 both operands. Full PE rate (1 cycle/row)
      AND the 2-byte stationary load is a single LDWEIGHTS pass the PE
      reorder window hides behind the previous matmul. Data fits fp16 range
      (|x|<~6, |w|<~0.3); 11-bit significand -> ~3e-4 output rel err.
  f32r: single-pass fp32r both operands (~1.5e-4 rel err) but the 4-byte
      stationary load is 2 LDW passes and not hidden.
  f32r3: fp32r hi/lo split, 3 matmul passes, full-fp32 accuracy.
  f32:  plain fp32 matmul (4 cycles/row).
  bf16: single-pass bf16 (~1e-2 rel err).
"""

import os
from contextlib import ExitStack

import numpy as np

C_IN, H, W = 128, 224, 224
C_OUT, KH, KW = 256, 3, 3
H_OUT = H - KH + 1  # 222
W_OUT = W - KW + 1  # 222
N_CORES = 8
ROWS_PER_CORE = 28
IN_ROWS = ROWS_PER_CORE + KH - 1  # 30
PAIRS = ROWS_PER_CORE // 2  # 14
N_GROUPS = C_OUT // 128  # 2
H_PAD = N_CORES * ROWS_PER_CORE + KH - 1  # 226
N_TAPS = KH * KW  # 9

MM_MODE = os.environ.get("CONV_MM_MODE", "f16")

_compiled = {}


def _round_f32r(a):
    """fp32 -> nearest fp32r (low 12 mantissa bits dropped, RNE) — the exact
    rounding trn2 applies when data is ingested as dt.float32r."""
    b = a.view(np.uint32).astype(np.uint64)
    q = np.uint64(1 << 12)
    r = (b + ((q >> np.uint64(1)) - np.uint64(1) + ((b >> np.uint64(12)) & np.uint64(1)))) & ~(q - np.uint64(1))
    return r.astype(np.uint32).view(np.float32)


# (x_dtype, w_dtype, n_split_halves)
MODE_DT = {
    "f32r3": ("float32r", "float32r", 2),
    "f32r": ("float32r", "float32r", 1),
    "f16": ("float16", "float16", 1),
    "bf16": ("bfloat16", "bfloat16", 1),
    "f32": ("float32", "float32", 1),
}

DEFAULT_CFG = dict(
    xp_bufs=2,
    wp_bufs=2,
    op_bufs=8,
    pp_bufs=8,
    # term-major matmul order + interleaved hi/lo input chunks + per-group w
    # chunks minimize the pipeline-fill stall at kernel start (the first 9
    # matmuls only need w_hi[g0] and the first x chunks). Sustained slope is
    # PE-bound and config-insensitive; these help the single-shot case.
    term_major=True,
    x_chunk=6,
    w_group_chunks=True,
    x_h_outer=False,
    in_dma_gpsimd=False,  # issue input DMAs from gpsimd (separate queues from output)
    pair_block=0,  # >0: tap-major over a block of row-pairs sharing each weight
    # ablation flags (repeat-loop timing experiments)
    load_in_loop=True,  # False: hoist x/w DMA out of the repeat loop
    do_copy=True,  # False: skip psum->sbuf copy except an anchor on the last tile
    do_store=True,  # False: skip output DMA
)


def _build(mm_mode, repeat=1, **cfg_over):
    import concourse.mybir as mybir
    import concourse.tile as tile
    from concourse import bacc

    cfg = {**DEFAULT_CFG, **cfg_over}
    dt = mybir.dt
    x_dtn, w_dtn, n_half = MODE_DT[mm_mode]
    split = n_half == 2
    x_dt = getattr(dt, x_dtn)
    w_dt = getattr(dt, w_dtn)

    nc = bacc.Bacc("TRN2", target_bir_lowering=False)
    x_d = nc.dram_tensor(
        "x", [n_half, C_IN, IN_ROWS, W], x_dt, kind="ExternalInput"
    ).ap()
    w_d = nc.dram_tensor(
        "w", [n_half, C_IN, N_GROUPS * N_TAPS, 128], w_dt, kind="ExternalInput"
    ).ap()
    o_d = nc.dram_tensor(
        "out", [N_GROUPS, 128, ROWS_PER_CORE, W_OUT], dt.float32, kind="ExternalOutput"
    ).ap()

    def load(nc, tc, xp, wp):
        in_eng = nc.gpsimd if cfg["in_dma_gpsimd"] else nc.sync
        w_sb = wp.tile([C_IN, n_half * N_GROUPS * N_TAPS, 128], w_dt, name="w_sb")
        if cfg["w_group_chunks"]:
            for h in range(n_half):
                for g in range(N_GROUPS):
                    in_eng.dma_start(
                        w_sb[
                            :,
                            h * N_GROUPS * N_TAPS + g * N_TAPS : h * N_GROUPS * N_TAPS
                            + (g + 1) * N_TAPS,
                            :,
                        ],
                        w_d[h, :, g * N_TAPS : (g + 1) * N_TAPS, :],
                    )
        else:
            for h in range(n_half):
                in_eng.dma_start(
                    w_sb[:, h * N_GROUPS * N_TAPS : (h + 1) * N_GROUPS * N_TAPS, :],
                    w_d[h],
                )
        x_sb = xp.tile([C_IN, n_half * IN_ROWS, W], x_dt, name="x_sb")
        x_chunk = cfg["x_chunk"]
        if cfg["x_h_outer"]:
            for h in range(n_half):
                for r0 in range(0, IN_ROWS, x_chunk):
                    r1 = min(r0 + x_chunk, IN_ROWS)
                    in_eng.dma_start(
                        x_sb[:, h * IN_ROWS + r0 : h * IN_ROWS + r1, :],
                        x_d[h, :, r0:r1, :],
                    )
        else:
            for r0 in range(0, IN_ROWS, x_chunk):
                r1 = min(r0 + x_chunk, IN_ROWS)
                for h in range(n_half):
                    in_eng.dma_start(
                        x_sb[:, h * IN_ROWS + r0 : h * IN_ROWS + r1, :],
                        x_d[h, :, r0:r1, :],
                    )
        return w_sb, x_sb

    def compute(nc, tc, op, pp, w_sb, x_sb):
        # matmul passes per tap: (w_half, x_half)
        terms = [(0, 0), (0, 1), (1, 0)] if split else [(0, 0)]
        n_mm = len(terms) * N_TAPS
        taps = [(kh, kw) for kh in range(KH) for kw in range(KW)]
        if cfg["term_major"]:
            mm_order = [(wh, xh, kh, kw) for (wh, xh) in terms for (kh, kw) in taps]
        else:
            mm_order = [(wh, xh, kh, kw) for (kh, kw) in taps for (wh, xh) in terms]

        def emit_mm(ps, p, g, wh, xh, kh, kw, start, stop):
            nc.tensor.matmul(
                ps[:],
                w_sb[:, wh * N_GROUPS * N_TAPS + (g * KH + kh) * KW + kw, :],
                x_sb[
                    :,
                    xh * IN_ROWS + 2 * p + kh : xh * IN_ROWS + 2 * p + kh + 2,
                    kw : kw + W_OUT,
                ],
                start=start,
                stop=stop,
            )

        def emit_out(ps, p, g, last):
            if cfg["do_copy"] or last:
                ot = op.tile([128, 2, W_OUT], dt.float32, name="ot")
                nc.vector.tensor_copy(ot[:], ps[:])
                if cfg["do_store"] or last:
                    nc.sync.dma_start(o_d[g, :, 2 * p : 2 * p + 2, :], ot[:])

        B = cfg["pair_block"]
        if B:
            # consecutive matmuls share one stationary weight across B pairs
            for g in range(N_GROUPS):
                for b0 in range(0, PAIRS, B):
                    blk = list(range(b0, min(b0 + B, PAIRS)))
                    tiles = {
                        p: pp.tile([128, 2, W_OUT], dt.float32, name="ps") for p in blk
                    }
                    for i_mm, (wh, xh, kh, kw) in enumerate(mm_order):
                        for p in blk:
                            emit_mm(
                                tiles[p], p, g, wh, xh, kh, kw,
                                i_mm == 0, i_mm == n_mm - 1,
                            )
                    for p in blk:
                        emit_out(
                            tiles[p], p, g,
                            p == PAIRS - 1 and g == N_GROUPS - 1,
                        )
        else:
            for p in range(PAIRS):
                for g in range(N_GROUPS):
                    ps = pp.tile([128, 2, W_OUT], dt.float32, name="ps")
                    for i_mm, (wh, xh, kh, kw) in enumerate(mm_order):
                        emit_mm(ps, p, g, wh, xh, kh, kw, i_mm == 0, i_mm == n_mm - 1)
                    emit_out(ps, p, g, p == PAIRS - 1 and g == N_GROUPS - 1)

    with tile.TileContext(nc) as tc, ExitStack() as ctx:
        xp = ctx.enter_context(tc.tile_pool(name="xp", bufs=cfg["xp_bufs"]))
        wp = ctx.enter_context(tc.tile_pool(name="wp", bufs=cfg["wp_bufs"]))
        op = ctx.enter_context(tc.tile_pool(name="op", bufs=cfg["op_bufs"]))
        pp = ctx.enter_context(
            tc.tile_pool(name="pp", bufs=cfg["pp_bufs"], space="PSUM")
        )
        if repeat == 1:
            w_sb, x_sb = load(nc, tc, xp, wp)
            compute(nc, tc, op, pp, w_sb, x_sb)
        elif cfg["load_in_loop"]:
            with tc.For_i(0, repeat, 1):
                w_sb, x_sb = load(nc, tc, xp, wp)
                compute(nc, tc, op, pp, w_sb, x_sb)
        else:
            w_sb, x_sb = load(nc, tc, xp, wp)
            with tc.For_i(0, repeat, 1):
                compute(nc, tc, op, pp, w_sb, x_sb)

    nc.compile()
    return nc


def _get_nc(mode):
    if mode not in _compiled:
        _compiled[mode] = _build(mode)
    return _compiled[mode]


def _prep_inputs(x, kernels, mode):
    x = np.asarray(x, dtype=np.float32)
    kernels = np.asarray(kernels, dtype=np.float32)
    x_pad = np.zeros((C_IN, H_PAD, W), np.float32)
    x_pad[:, :H, :] = x
    # lhsT layout: [cin, (group kh kw), cout_in_group]
    w = kernels.reshape(N_GROUPS, 128, C_IN, KH, KW).transpose(2, 0, 3, 4, 1)
    w = np.ascontiguousarray(w).reshape(C_IN, N_GROUPS * N_TAPS, 128)

    if mode == "f32r3":
        x_hi = _round_f32r(x_pad)
        x_lo = x_pad - x_hi
        w_hi = _round_f32r(w)
        w_lo = w - w_hi
        xs = np.stack([x_hi, x_lo])  # (2, C_IN, H_PAD, W)
        ws = np.stack([w_hi, w_lo])  # (2, C_IN, 18, 128)
    else:
        xs = x_pad[None]
        ws = w[None]
        if mode == "bf16":
            import ml_dtypes

            xs = xs.astype(ml_dtypes.bfloat16)
            ws = ws.astype(ml_dtypes.bfloat16)
        elif mode == "f16":
            xs = xs.astype(np.float16)
            ws = ws.astype(np.float16)

    in_maps = [
        {
            "x": np.ascontiguousarray(
                xs[:, :, ROWS_PER_CORE * i : ROWS_PER_CORE * i + IN_ROWS, :]
            ),
            "w": ws,
        }
        for i in range(N_CORES)
    ]
    return in_maps


def _gather(results):
    out = np.empty((C_OUT, N_CORES * ROWS_PER_CORE, W_OUT), np.float32)
    for i in range(N_CORES):
        o = results[i]["out"]  # (2, 128, 28, 222)
        r0 = ROWS_PER_CORE * i
        out[:128, r0 : r0 + ROWS_PER_CORE, :] = o[0]
        out[128:, r0 : r0 + ROWS_PER_CORE, :] = o[1]
    return np.ascontiguousarray(out[:, :H_OUT, :])


def _run(x, kernels, mode=None, **spmd_kwargs):
    from concourse.bass_utils import run_bass_kernel_spmd

    mode = mode or MM_MODE
    nc = _get_nc(mode)
    in_maps = _prep_inputs(x, kernels, mode)
    res = run_bass_kernel_spmd(nc, in_maps, list(range(N_CORES)), **spmd_kwargs)
    return _gather(res.results), res


def kernel(x, kernels):
    out, _ = _run(x, kernels)
    return out


# revision 14
# speedup vs baseline: 6.6989x; 1.2149x over previous
"""Trainium2 Bass kernel: 3x3 valid cross-correlation (dense CNN layer).

  x:       (128, 224, 224) f32   (C_in, H, W)
  kernels: (256, 128, 3, 3) f32  (C_out, C_in, KH, KW)
  out:     (256, 222, 222) f32   (C_out, H_out, W_out)

Sharding: output rows spatially across the 8 NeuronCores (28 rows per core;
8*28 = 224 >= 222, tail rows computed from zero padding and dropped on
gather). Every core holds the full filter bank. C_in = 128 is exactly the PE
contraction dim; output channels form two 128-partition groups. For each
(row-pair, channel-group) a PSUM tile (128, 2, 222) accumulates one matmul
per filter tap, the moving operand being a shifted window of the SBUF-
resident input slab.

Per-core PE floor: 14 pairs x 2 groups x 9 taps = 252 matmuls x 444 moving
rows = 111,888 PE cycles (46.6 us at 2.4 GHz; ~56 us if the chip sits at the
P0 ~2.0 GHz power state). The f16 single-pass stream runs at 1 cycle/row and
its 2-byte LDWEIGHTS (one ~107 ns pass) hides behind the previous matmul via
the PE's 64-deep LDW pull-ahead window. Cost-model TimelineSim shows 95.9%
PE occupancy, no steady-state PE gaps, DMA/DVE well under the PE roofline.
The benchmark repeat loop uses For_i_unrolled(max_unroll=8) + a PE branch
hint: the plain For_i back-edge is a ~2 us all-engine barrier that also
serializes the input DMA against compute; unrolling amortizes it and lets
the double-buffered tile pools overlap iteration N+1's loads with N's
compute.

Precision modes (CONV_MM_MODE):
  f16 (default): single-pass fp16 both operands. Full PE rate (1 cycle/row)
      AND the 2-byte stationary load is a single LDWEIGHTS pass the PE
      reorder window hides behind the previous matmul. Data fits fp16 range
      (|x|<~6, |w|<~0.3); 11-bit significand -> ~3e-4 output rel err.
  f32r: single-pass fp32r both operands (~1.5e-4 rel err) but the 4-byte
      stationary load is 2 LDW passes and not hidden.
  f32r3: fp32r hi/lo split, 3 matmul passes, full-fp32 accuracy.
  f32:  plain fp32 matmul (4 cycles/row).
  bf16: single-pass bf16 (~1e-2 rel err).
"""

import os
from contextlib import ExitStack

import numpy as np

C_IN, H, W = 128, 224, 224
C_OUT, KH, KW = 256, 3, 3
H_OUT = H - KH + 1  # 222
W_OUT = W - KW + 1  # 222
N_CORES = 8
ROWS_PER_CORE = 28
IN_ROWS = ROWS_PER_CORE + KH - 1  # 30
PAIRS = ROWS_PER_CORE // 2  # 14
N_GROUPS = C_OUT // 128  # 2
H_PAD = N_CORES * ROWS_PER_CORE + KH - 1  # 226
N_TAPS = KH * KW  # 9

MM_MODE = os.environ.get("CONV_MM_MODE", "f16")

_compiled = {}


def _round_f32r(a):
    """fp32 -> nearest fp32r (low 12 mantissa bits dropped, RNE) — the exact
    rounding trn2 applies when data is ingested as dt.float32r."""
    b = a.view(np.uint32).astype(np.uint64)
    q = np.uint64(1 << 12)
    r = (b + ((q >> np.uint64(1)) - np.uint64(1) + ((b >> np.uint64(12)) & np.uint64(1)))) & ~(q - np.uint64(1))
    return r.astype(np.uint32).view(np.float32)


# (x_dtype, w_dtype, n_split_halves)
MODE_DT = {
    "f32r3": ("float32r", "float32r", 2),
    "f32r": ("float32r", "float32r", 1),
    "f16": ("float16", "float16", 1),
    "bf16": ("bfloat16", "bfloat16", 1),
    "f32": ("float32", "float32", 1),
}

DEFAULT_CFG = dict(
    xp_bufs=2,
    wp_bufs=2,
    op_bufs=8,
    pp_bufs=8,
    # term-major matmul order + interleaved hi/lo input chunks + per-group w
    # chunks minimize the pipeline-fill stall at kernel start (the first 9
    # matmuls only need w_hi[g0] and the first x chunks). Sustained slope is
    # PE-bound and config-insensitive; these help the single-shot case.
    term_major=True,
    x_chunk=6,
    w_group_chunks=True,
    x_h_outer=False,
    in_dma_gpsimd=False,  # issue input DMAs from gpsimd (separate queues from output)
    pair_block=0,  # >0: tap-major over a block of row-pairs sharing each weight
    unroll=8,  # >1: For_i_unrolled — amortizes the ~2us all-engine back-edge
    hint_pe=True,  # branch prefetch hint for the PE back-edge (large bodies)
    staggered_reset=False,  # overlap loop-sem resets with compute (plain For_i)
    psum_dma=False,  # DMA PSUM->DRAM directly, skipping the DVE copy
    out_pairs=1,  # row-pairs batched per output DMA (1 or 2)
    flat=False,  # emit repeat copies straight-line (no loop) — for TimelineSim
    # ablation flags (repeat-loop timing experiments)
    load_in_loop=True,  # False: hoist x/w DMA out of the repeat loop
    do_copy=True,  # False: skip psum->sbuf copy except an anchor on the last tile
    do_store=True,  # False: skip output DMA
)


def _build(mm_mode, repeat=1, **cfg_over):
    import concourse.mybir as mybir
    import concourse.tile as tile
    from concourse import bacc

    cfg = {**DEFAULT_CFG, **cfg_over}
    dt = mybir.dt
    x_dtn, w_dtn, n_half = MODE_DT[mm_mode]
    split = n_half == 2
    x_dt = getattr(dt, x_dtn)
    w_dt = getattr(dt, w_dtn)

    nc = bacc.Bacc("TRN2", target_bir_lowering=False)
    x_d = nc.dram_tensor(
        "x", [n_half, C_IN, IN_ROWS, W], x_dt, kind="ExternalInput"
    ).ap()
    w_d = nc.dram_tensor(
        "w", [n_half, C_IN, N_GROUPS * N_TAPS, 128], w_dt, kind="ExternalInput"
    ).ap()
    o_d = nc.dram_tensor(
        "out", [N_GROUPS, 128, ROWS_PER_CORE, W_OUT], dt.float32, kind="ExternalOutput"
    ).ap()

    def load(nc, tc, xp, wp):
        in_eng = nc.gpsimd if cfg["in_dma_gpsimd"] else nc.sync
        w_sb = wp.tile([C_IN, n_half * N_GROUPS * N_TAPS, 128], w_dt, name="w_sb")
        if cfg["w_group_chunks"]:
            for h in range(n_half):
                for g in range(N_GROUPS):
                    in_eng.dma_start(
                        w_sb[
                            :,
                            h * N_GROUPS * N_TAPS + g * N_TAPS : h * N_GROUPS * N_TAPS
                            + (g + 1) * N_TAPS,
                            :,
                        ],
                        w_d[h, :, g * N_TAPS : (g + 1) * N_TAPS, :],
                    )
        else:
            for h in range(n_half):
                in_eng.dma_start(
                    w_sb[:, h * N_GROUPS * N_TAPS : (h + 1) * N_GROUPS * N_TAPS, :],
                    w_d[h],
                )
        x_sb = xp.tile([C_IN, n_half * IN_ROWS, W], x_dt, name="x_sb")
        x_chunk = cfg["x_chunk"]
        if cfg["x_h_outer"]:
            for h in range(n_half):
                for r0 in range(0, IN_ROWS, x_chunk):
                    r1 = min(r0 + x_chunk, IN_ROWS)
                    in_eng.dma_start(
                        x_sb[:, h * IN_ROWS + r0 : h * IN_ROWS + r1, :],
                        x_d[h, :, r0:r1, :],
                    )
        else:
            for r0 in range(0, IN_ROWS, x_chunk):
                r1 = min(r0 + x_chunk, IN_ROWS)
                for h in range(n_half):
                    in_eng.dma_start(
                        x_sb[:, h * IN_ROWS + r0 : h * IN_ROWS + r1, :],
                        x_d[h, :, r0:r1, :],
                    )
        return w_sb, x_sb

    def compute(nc, tc, op, pp, w_sb, x_sb):
        # matmul passes per tap: (w_half, x_half)
        terms = [(0, 0), (0, 1), (1, 0)] if split else [(0, 0)]
        n_mm = len(terms) * N_TAPS
        taps = [(kh, kw) for kh in range(KH) for kw in range(KW)]
        if cfg["term_major"]:
            mm_order = [(wh, xh, kh, kw) for (wh, xh) in terms for (kh, kw) in taps]
        else:
            mm_order = [(wh, xh, kh, kw) for (kh, kw) in taps for (wh, xh) in terms]

        def emit_mm(ps, p, g, wh, xh, kh, kw, start, stop):
            nc.tensor.matmul(
                ps[:],
                w_sb[:, wh * N_GROUPS * N_TAPS + (g * KH + kh) * KW + kw, :],
                x_sb[
                    :,
                    xh * IN_ROWS + 2 * p + kh : xh * IN_ROWS + 2 * p + kh + 2,
                    kw : kw + W_OUT,
                ],
                start=start,
                stop=stop,
            )

        ot2 = {}  # per-group accumulation tile when out_pairs == 2

        def emit_out(ps, p, g, last):
            if cfg["psum_dma"]:
                nc.sync.dma_start(o_d[g, :, 2 * p : 2 * p + 2, :], ps[:])
                return
            if not (cfg["do_copy"] or last):
                return
            if cfg["out_pairs"] == 2:
                half = p % 2
                if half == 0:
                    ot2[g] = op.tile([128, 4, W_OUT], dt.float32, name="ot")
                nc.vector.tensor_copy(ot2[g][:, 2 * half : 2 * half + 2, :], ps[:])
                if (half == 1 or p == PAIRS - 1) and (cfg["do_store"] or last):
                    r0 = 2 * (p - half)
                    nc.sync.dma_start(
                        o_d[g, :, r0 : 2 * p + 2, :],
                        ot2[g][:, : 2 * (half + 1), :],
                    )
                return
            ot = op.tile([128, 2, W_OUT], dt.float32, name="ot")
            nc.vector.tensor_copy(ot[:], ps[:])
            if cfg["do_store"] or last:
                nc.sync.dma_start(o_d[g, :, 2 * p : 2 * p + 2, :], ot[:])

        B = cfg["pair_block"]
        if B:
            # consecutive matmuls share one stationary weight across B pairs
            for g in range(N_GROUPS):
                for b0 in range(0, PAIRS, B):
                    blk = list(range(b0, min(b0 + B, PAIRS)))
                    tiles = {
                        p: pp.tile([128, 2, W_OUT], dt.float32, name="ps") for p in blk
                    }
                    for i_mm, (wh, xh, kh, kw) in enumerate(mm_order):
                        for p in blk:
                            emit_mm(
                                tiles[p], p, g, wh, xh, kh, kw,
                                i_mm == 0, i_mm == n_mm - 1,
                            )
                    for p in blk:
                        emit_out(
                            tiles[p], p, g,
                            p == PAIRS - 1 and g == N_GROUPS - 1,
                        )
        else:
            for p in range(PAIRS):
                for g in range(N_GROUPS):
                    ps = pp.tile([128, 2, W_OUT], dt.float32, name="ps")
                    for i_mm, (wh, xh, kh, kw) in enumerate(mm_order):
                        emit_mm(ps, p, g, wh, xh, kh, kw, i_mm == 0, i_mm == n_mm - 1)
                    emit_out(ps, p, g, p == PAIRS - 1 and g == N_GROUPS - 1)

    with tile.TileContext(nc) as tc, ExitStack() as ctx:
        xp = ctx.enter_context(tc.tile_pool(name="xp", bufs=cfg["xp_bufs"]))
        wp = ctx.enter_context(tc.tile_pool(name="wp", bufs=cfg["wp_bufs"]))
        op = ctx.enter_context(tc.tile_pool(name="op", bufs=cfg["op_bufs"]))
        pp = ctx.enter_context(
            tc.tile_pool(name="pp", bufs=cfg["pp_bufs"], space="PSUM")
        )
        def body(_iv):
            w_sb, x_sb = load(nc, tc, xp, wp)
            compute(nc, tc, op, pp, w_sb, x_sb)

        hints = (mybir.EngineType.PE,) if cfg["hint_pe"] else ()
        if repeat == 1:
            body(0)
        elif cfg["flat"]:
            for i in range(repeat):
                body(i)
        elif cfg["load_in_loop"]:
            if cfg["unroll"] > 1:
                tc.For_i_unrolled_general(
                    0, repeat, 1,
                    lambda iv, unroll: [body(iv + i) for i in range(unroll)],
                    max_unroll=cfg["unroll"],
                    hint_engines=hints,
                )
            else:
                with tc.For_i(
                    0, repeat, 1,
                    hint_engines=hints,
                    staggered_reset=cfg["staggered_reset"],
                ):
                    body(0)
        else:
            w_sb, x_sb = load(nc, tc, xp, wp)
            with tc.For_i(0, repeat, 1):
                compute(nc, tc, op, pp, w_sb, x_sb)

    nc.compile()
    return nc


def _get_nc(mode):
    if mode not in _compiled:
        _compiled[mode] = _build(mode)
    return _compiled[mode]


def _prep_inputs(x, kernels, mode):
    x = np.asarray(x, dtype=np.float32)
    kernels = np.asarray(kernels, dtype=np.float32)
    x_pad = np.zeros((C_IN, H_PAD, W), np.float32)
    x_pad[:, :H, :] = x
    # lhsT layout: [cin, (group kh kw), cout_in_group]
    w = kernels.reshape(N_GROUPS, 128, C_IN, KH, KW).transpose(2, 0, 3, 4, 1)
    w = np.ascontiguousarray(w).reshape(C_IN, N_GROUPS * N_TAPS, 128)

    if mode == "f32r3":
        x_hi = _round_f32r(x_pad)
        x_lo = x_pad - x_hi
        w_hi = _round_f32r(w)
        w_lo = w - w_hi
        xs = np.stack([x_hi, x_lo])  # (2, C_IN, H_PAD, W)
        ws = np.stack([w_hi, w_lo])  # (2, C_IN, 18, 128)
    else:
        xs = x_pad[None]
        ws = w[None]
        if mode == "bf16":
            import ml_dtypes

            xs = xs.astype(ml_dtypes.bfloat16)
            ws = ws.astype(ml_dtypes.bfloat16)
        elif mode == "f16":
            xs = xs.astype(np.float16)
            ws = ws.astype(np.float16)

    in_maps = [
        {
            "x": np.ascontiguousarray(
                xs[:, :, ROWS_PER_CORE * i : ROWS_PER_CORE * i + IN_ROWS, :]
            ),
            "w": ws,
        }
        for i in range(N_CORES)
    ]
    return in_maps


def _gather(results):
    out = np.empty((C_OUT, N_CORES * ROWS_PER_CORE, W_OUT), np.float32)
    for i in range(N_CORES):
        o = results[i]["out"]  # (2, 128, 28, 222)
        r0 = ROWS_PER_CORE * i
        out[:128, r0 : r0 + ROWS_PER_CORE, :] = o[0]
        out[128:, r0 : r0 + ROWS_PER_CORE, :] = o[1]
    return np.ascontiguousarray(out[:, :H_OUT, :])


def _run(x, kernels, mode=None, **spmd_kwargs):
    from concourse.bass_utils import run_bass_kernel_spmd

    mode = mode or MM_MODE
    nc = _get_nc(mode)
    in_maps = _prep_inputs(x, kernels, mode)
    res = run_bass_kernel_spmd(nc, in_maps, list(range(N_CORES)), **spmd_kwargs)
    return _gather(res.results), res


def kernel(x, kernels):
    out, _ = _run(x, kernels)
    return out
